# revision 1
# baseline (speedup 1.0000x reference)
# GAT message-passing kernel for 8 TRN2 NeuronCores (raw bass, SPMD).
#
# Strategy (dst-sharded, no collectives):
#  - Edges bucketed by dst into 8 cores x 16 windows (128 dsts each), padded
#    to T 128-edge tiles per window.
#  - Per tile: indirect-gather bf16 feat rows by src, PE-transpose, matmul
#    against [fc_w.T | W_el] (W_el = attn_l folded into fc_w so el comes from
#    the same matmul), edge coefficients exp(leaky(el+er)) via ACT, weighted
#    segment-sum via one-hot matmuls into PSUM per window.
#  - er/res per window from local feat.T columns; POI branch gathered from a
#    host-packed [cat|coeff|count] table; epilogue ELU + rescale on DVE/ACT.
import dataclasses
import numpy as np
import ml_dtypes

import concourse.bass as bass
from concourse import mybir

INC_G = 16   # sem inc per feat-row indirect gather (sim-calibrated)
INC_OH = 64  # sem inc per onehot-pair HWDGE load
INC_P = 16   # sem inc per poi indirect gather
BF16 = mybir.dt.bfloat16
F32 = mybir.dt.float32
I32 = mybir.dt.int32


@dataclasses.dataclass
class Cfg:
    n_src: int = 65536
    n_dst: int = 16384
    n_edges: int = 262144
    in_dim: int = 256          # must be 2*128
    hid: int = 128
    heads: int = 4
    n_poi: int = 100000
    k: int = 50
    neg_slope: float = 0.2
    cores: int = 8

    @property
    def f(self):  # total output features
        return self.heads * self.hid

    @property
    def dsts_per_core(self):
        return self.n_dst // self.cores

    @property
    def windows(self):
        return self.dsts_per_core // 128


def host_prep(inputs: dict, cfg: Cfg):
    """Build per-core input maps. Index prep / layout / dtype casts only."""
    bf = ml_dtypes.bfloat16
    feat = np.asarray(inputs["feat"], np.float32)
    poi_cat = np.asarray(inputs["poi_cat_feat"], np.float32)
    poi_coe = np.asarray(inputs["poi_coeff"], np.float32)
    fc_w = np.asarray(inputs["fc_w"], np.float32)
    attn_l = np.asarray(inputs["attn_l"], np.float32)
    attn_r = np.asarray(inputs["attn_r"], np.float32)
    bias_p = np.asarray(inputs["bias_p"], np.float32)
    res_w = np.asarray(inputs["res_w"], np.float32)
    w_w = np.asarray(inputs["w_w"], np.float32)
    src_idx = np.asarray(inputs["src_idx"]).astype(np.int64)
    dst_idx = np.asarray(inputs["dst_idx"]).astype(np.int64)
    out_nodes = np.asarray(inputs["output_nodes"]).astype(np.int64)
    ncounts = np.asarray(inputs["neighbor_counts"]).astype(np.int64)

    H, HID, IN, F, K = cfg.heads, cfg.hid, cfg.in_dim, cfg.f, cfg.k
    W, DPC = cfg.windows, cfg.dsts_per_core

    # --- weight folding (weights-only math) ---
    # el[s,h] = sum_k feat[s,k] * W_el[k,h];  W_el[k,h] = sum_f fc_w[h*HID+f,k]*attn_l[h,f]
    W_el = np.einsum("hfk,hf->kh", fc_w.reshape(H, HID, IN), attn_l)  # [IN, H]
    W_er = np.einsum("hfk,hf->kh", fc_w.reshape(H, HID, IN), attn_r)  # [IN, H]

    fcwel = np.concatenate([fc_w.T, W_el], axis=1)            # [IN, F+H]
    reswer = np.concatenate([res_w.T, W_er], axis=1)          # [IN, F+H]
    # pack [IN, X] as [128, 2*X]: chunk c -> cols [c*X:(c+1)*X]
    def pack_k(a):
        X = a.shape[1]
        return np.concatenate([a[0:128], a[128:256]], axis=1).astype(bf)

    fcwel_p = pack_k(fcwel)                                   # [128, 2*(F+H)]
    reswer_p = pack_k(reswer)                                 # [128, 2*(F+H)]
    wwT_p = pack_k(w_w.T)                                     # [128, 2*HID]

    featb = np.ascontiguousarray(feat).astype(bf)             # [n_src, IN]
    featT = feat.T.astype(bf)                                 # [IN, n_dst slice used]
    bias_b = np.tile(bias_p[None, :], (128, 1)).astype(np.float32)   # [128, F]
    iota = np.zeros((128, 66), np.float32)
    iota[:, :64] = np.arange(64, dtype=np.float32)[None, :]
    iota[:, 64] = -0.5
    ident = np.eye(128, dtype=bf)

    # POI combined table: [cat(IN) | coeff(K..pad64) | count | pad] -> 320 f32
    PW = IN + 64
    poitab = np.zeros((cfg.n_poi, PW), np.float32)
    poitab[:, :IN] = poi_cat
    poitab[:, IN:IN + K] = poi_coe
    poitab[:, IN + 56] = ncounts.astype(np.float32)           # count at col IN+56
    assert K <= 56

    # --- edge bucketing ---
    order = np.argsort(dst_idx, kind="stable")
    ds = dst_idx[order]
    # counts per (core, window)
    win_of = ds // 128                                        # global window id
    n_win_glob = cfg.n_dst // 128
    cnt = np.bincount(win_of, minlength=n_win_glob)
    T = int(np.ceil(cnt.max() / 128))
    starts = np.zeros(n_win_glob + 1, np.int64)
    np.cumsum(cnt, out=starts[1:])

    in_maps = []
    for c in range(cfg.cores):
        gidx = np.zeros((128, W * T), np.int32)
        dloc = np.full((W * T, 128), 255, np.int32)           # 255 = pad slot
        for w in range(W):
            gw = c * W + w
            e0, e1 = starts[gw], starts[gw + 1]
            sel = order[e0:e1]
            nme = e1 - e0
            s_pad = np.zeros(T * 128, np.int64)
            d_pad = np.full(T * 128, 255, np.int64)
            s_pad[:nme] = src_idx[sel]
            d_pad[:nme] = dst_idx[sel] % 128
            for t in range(T):
                n = w * T + t
                gidx[:, n] = s_pad[t * 128:(t + 1) * 128]
                dloc[n] = d_pad[t * 128:(t + 1) * 128]
        # one-hot [NT,128,128] + transpose, packed as [NT,128,256]
        oh = (dloc[:, :, None] == np.arange(128)[None, None, :])
        ohcomb = np.zeros((W * T, 128, 256), bf)
        ohcomb[:, :, :128] = oh.astype(bf)
        ohcomb[:, :, 128:] = np.transpose(oh, (0, 2, 1)).astype(bf)

        poix = np.zeros((128, 16), np.int32)
        poix[:, :W] = out_nodes[c * DPC:(c + 1) * DPC].reshape(W, 128).T

        fTd = featT[:, c * DPC:(c + 1) * DPC]                 # [IN, DPC]
        fTd_p = np.concatenate([fTd[0:128], fTd[128:256]], axis=1).astype(bf)  # [128, 2*DPC]

        in_maps.append(dict(
            featb=featb, poitab=poitab,
            gidx=gidx, ohcomb=np.ascontiguousarray(ohcomb), poix=poix,
            fcwel=fcwel_p, reswer=reswer_p, wwt=wwT_p, ftd=fTd_p,
            biasb=bias_b, iota=iota, ident=ident,
        ))
    return in_maps, T


def build_nc(cfg: Cfg, T: int):
    H, HID, IN, F, K = cfg.heads, cfg.hid, cfg.in_dim, cfg.f, cfg.k
    NEG = cfg.neg_slope
    W, DPC = cfg.windows, cfg.dsts_per_core
    NT = W * T
    FH = F + H                     # 516
    GB, OH = 4, 4                  # gather / onehot slot counts
    IG = INC_G
    IP = INC_P
    IO = INC_OH
    PW = IN + 64                   # poi row width (f32)

    nc = bass.Bass(num_devices=cfg.cores)
    # DRAM inputs
    featb = nc.dram_tensor("featb", [cfg.n_src, IN], BF16, kind="ExternalInput")
    poitab = nc.dram_tensor("poitab", [cfg.n_poi, PW], F32, kind="ExternalInput")
    gidx = nc.dram_tensor("gidx", [128, NT], I32, kind="ExternalInput")
    ohcomb = nc.dram_tensor("ohcomb", [NT, 128, 256], BF16, kind="ExternalInput")
    poix = nc.dram_tensor("poix", [128, 16], I32, kind="ExternalInput")
    fcwel = nc.dram_tensor("fcwel", [128, 2 * FH], BF16, kind="ExternalInput")
    reswer = nc.dram_tensor("reswer", [128, 2 * FH], BF16, kind="ExternalInput")
    wwt = nc.dram_tensor("wwt", [128, 2 * HID], BF16, kind="ExternalInput")
    ftd = nc.dram_tensor("ftd", [128, 2 * DPC], BF16, kind="ExternalInput")
    biasb = nc.dram_tensor("biasb", [128, F], F32, kind="ExternalInput")
    iota = nc.dram_tensor("iota", [128, 66], F32, kind="ExternalInput")
    ident = nc.dram_tensor("ident", [128, 128], BF16, kind="ExternalInput")
    out_d = nc.dram_tensor("out", [DPC, F], F32, kind="ExternalOutput")

    NPRE = 9   # preload DMA count (all but featb/poitab/ohcomb which stream)

    from contextlib import ExitStack
    ctx = ExitStack()
    idx_sb = ctx.enter_context(nc.sbuf_tensor([128, NT], I32))
    poix_sb = ctx.enter_context(nc.sbuf_tensor([128, 16], I32))
    fcwel_sb = ctx.enter_context(nc.sbuf_tensor([128, 2 * FH], BF16))
    reswer_sb = ctx.enter_context(nc.sbuf_tensor([128, 2 * FH], BF16))
    wwt_sb = ctx.enter_context(nc.sbuf_tensor([128, 2 * HID], BF16))
    ftd_sb = ctx.enter_context(nc.sbuf_tensor([128, 2 * DPC], BF16))
    bias_sb = ctx.enter_context(nc.sbuf_tensor([128, F], F32))
    iota_sb = ctx.enter_context(nc.sbuf_tensor([128, 66], F32))
    ident_sb = ctx.enter_context(nc.sbuf_tensor([128, 128], BF16))
    g_sb = ctx.enter_context(nc.sbuf_tensor([128, GB * IN], BF16))
    oh_sb = ctx.enter_context(nc.sbuf_tensor([128, OH * 256], BF16))
    gt_sb = ctx.enter_context(nc.sbuf_tensor([128, 2 * IN], BF16))
    erf_sb = ctx.enter_context(nc.sbuf_tensor([128, 2 * H], F32))
    e_sb = ctx.enter_context(nc.sbuf_tensor([128, 2 * H], F32))
    lr_sb = ctx.enter_context(nc.sbuf_tensor([128, 2 * H], F32))
    exp_sb = ctx.enter_context(nc.sbuf_tensor([128, 2 * H], F32))
    expb_sb = ctx.enter_context(nc.sbuf_tensor([128, 2 * H], BF16))
    erw_sb = ctx.enter_context(nc.sbuf_tensor([128, 2 * H], BF16))
    msgs_sb = ctx.enter_context(nc.sbuf_tensor([128, 2 * F], BF16))
    poi_sb = ctx.enter_context(nc.sbuf_tensor([128, 2 * PW], F32))
    cft_sb = ctx.enter_context(nc.sbuf_tensor([128, 2 * IN], BF16))
    s_sb = ctx.enter_context(nc.sbuf_tensor([128, 2], F32))
    mask_sb = ctx.enter_context(nc.sbuf_tensor([128, 64], F32))
    dinv_sb = ctx.enter_context(nc.sbuf_tensor([128, 2 * H], F32))
    z_sb = ctx.enter_context(nc.sbuf_tensor([128, F], F32))
    zn_sb = ctx.enter_context(nc.sbuf_tensor([128, F], F32))
    e1_sb = ctx.enter_context(nc.sbuf_tensor([128, F], F32))
    pos_sb = ctx.enter_context(nc.sbuf_tensor([128, F], F32))
    q_sb = ctx.enter_context(nc.sbuf_tensor([128, F], F32))
    fd_sb = ctx.enter_context(nc.sbuf_tensor([128, 2 * HID], F32))
    outs_sb = ctx.enter_context(nc.sbuf_tensor([128, 2 * F], F32))
    rst_ps = ctx.enter_context(nc.psum_tensor([128, F], F32))
    den_ps = ctx.enter_context(nc.psum_tensor([128, F], F32))
    resw_ps = ctx.enter_context(nc.psum_tensor([128, FH], F32))
    fc_ps = ctx.enter_context(nc.psum_tensor([128, F + 2 * H], F32))
    tpA_ps = ctx.enter_context(nc.psum_tensor([128, 1024], BF16))
    tpB_ps = ctx.enter_context(nc.psum_tensor([128, 1024], BF16))
    ld = ctx.enter_context(nc.semaphore())
    ohm = [ctx.enter_context(nc.semaphore(name=f"ohm{i}")) for i in range(4)]
    gsem = [ctx.enter_context(nc.semaphore(name=f"gs{i}")) for i in range(4)]
    gpoi = [ctx.enter_context(nc.semaphore(name=f"gp{i}")) for i in range(2)]
    PEW = ctx.enter_context(nc.semaphore())
    PET = ctx.enter_context(nc.semaphore())
    PE2 = ctx.enter_context(nc.semaphore())
    PED = ctx.enter_context(nc.semaphore())
    PETC = ctx.enter_context(nc.semaphore())
    PEP = ctx.enter_context(nc.semaphore())
    DVW = ctx.enter_context(nc.semaphore())
    DVT = ctx.enter_context(nc.semaphore())
    DVA = ctx.enter_context(nc.semaphore())
    DVM = ctx.enter_context(nc.semaphore())
    DVC1 = ctx.enter_context(nc.semaphore())
    DVC2 = ctx.enter_context(nc.semaphore())
    DVR1 = ctx.enter_context(nc.semaphore())
    DVR2 = ctx.enter_context(nc.semaphore())
    DVZ = ctx.enter_context(nc.semaphore())
    DVQ = ctx.enter_context(nc.semaphore())
    AC1 = ctx.enter_context(nc.semaphore())
    AC2 = ctx.enter_context(nc.semaphore())
    AC3 = ctx.enter_context(nc.semaphore())
    AC4 = ctx.enter_context(nc.semaphore())
    osem = [ctx.enter_context(nc.semaphore(name=f"os{i}")) for i in range(2)]
    block = ctx.enter_context(nc.Block())
    with ctx:
        tp = [tpA_ps, tpB_ps]

        def gpos(w, t):  # gather sequence position of tile (w,t); poi w at w*(T+1)
            return w * (T + 1) + 1 + t

        ohmark = {}

        @block.sync
        def _(sy):
            sy.dma_start(idx_sb[:, :], gidx[:, :]).then_inc(ld, 16)
            sy.dma_start(poix_sb[:, :], poix[:, :]).then_inc(ld, 16)
            sy.dma_start(fcwel_sb[:, :], fcwel[:, :]).then_inc(ld, 16)
            sy.dma_start(reswer_sb[:, :], reswer[:, :]).then_inc(ld, 16)
            sy.dma_start(wwt_sb[:, :], wwt[:, :]).then_inc(ld, 16)
            sy.dma_start(ftd_sb[:, :], ftd[:, :]).then_inc(ld, 16)
            sy.dma_start(bias_sb[:, :], biasb[:, :]).then_inc(ld, 16)
            sy.dma_start(iota_sb[:, :], iota[:, :]).then_inc(ld, 16)
            sy.dma_start(ident_sb[:, :], ident[:, :]).then_inc(ld, 16)
            for n in range(NT):
                if n >= OH:
                    sy.wait_ge(PED, n - OH + 1)
                s = n % OH
                sy.dma_start(oh_sb[:, s * 256:(s + 1) * 256], ohcomb[n, :, :]).then_inc(ohm[s], 16)
                ohmark[n] = (n // OH + 1) * 16

        gsem_val = [0]
        gmark = {}
        pmark = {}

        @block.gpsimd
        def _(gp):
            gp.wait_ge(ld, NPRE * 16)
            for w in range(W):
                if w >= 2:
                    gp.wait_ge(DVR2, w - 1)  # poi slot free
                pw = w % 2
                gp.indirect_dma_start(
                    out=poi_sb[:, pw * PW:(pw + 1) * PW], out_offset=None,
                    in_=poitab[:, :],
                    in_offset=bass.IndirectOffsetOnAxis(ap=poix_sb[:, w:w + 1], axis=0),
                ).then_inc(gpoi[w % 2], 16)
                pmark[w] = (w // 2 + 1) * 16
                for t in range(T):
                    n = w * T + t
                    if n >= GB:
                        gp.wait_ge(PET, n - GB + 1)  # gather slot free after transposes
                    s = n % GB
                    gp.indirect_dma_start(
                        out=g_sb[:, s * IN:(s + 1) * IN], out_offset=None,
                        in_=featb[:, :],
                        in_offset=bass.IndirectOffsetOnAxis(ap=idx_sb[:, n:n + 1], axis=0),
                    ).then_inc(gsem[s], 16)
                    gmark[(w, t)] = (n // GB + 1) * IG

        @block.tensor
        def _(pe):
            pe.wait_ge(ld, NPRE * 16)

            def erw_mm(w):
                # erw(w) into resw_ps[:, F:FH]; returns last matmul
                last = None
                for c in range(2):
                    last = pe.matmul(resw_ps[:, F:FH],
                              lhsT=ftd_sb[:, c * DPC + w * 128: c * DPC + (w + 1) * 128],
                              rhs=reswer_sb[:, c * FH + F: c * FH + FH],
                              start=(c == 0), stop=(c == 1))
                return last

            erw_mm(0).then_inc(PEW, 1)
            for w in range(W):
                pw = w % 2
                for t in range(T):
                    n = w * T + t
                    s2 = n % 2
                    sg = n % GB
                    so = n % OH
                    pe.wait_ge(gsem[n % GB], gmark[(w, t)])
                    if t == 0 and w > 0:
                        pe.wait_ge(DVC2, w)      # tp bank shared with tpc of prev window
                    # transposes of gathered tile (bf16 -> psum bf16)
                    pe.transpose(tp[s2][:, 0:128], g_sb[:, sg * IN: sg * IN + 128], ident_sb[:, :])
                    pe.transpose(tp[s2][:, 128:256], g_sb[:, sg * IN + 128: (sg + 1) * IN], ident_sb[:, :]).then_inc(PET, 1)
                    pe.wait_ge(DVT, n + 1)
                    # fc + el
                    for c in range(2):
                        lhs = gt_sb[:, s2 * IN + c * 128: s2 * IN + (c + 1) * 128]
                        pe.matmul(fc_ps[:, 0:F], lhsT=lhs,
                                  rhs=fcwel_sb[:, c * FH: c * FH + F],
                                  start=(c == 0), stop=(c == 1))
                        pe.matmul(fc_ps[:, F:F + H], lhsT=lhs,
                                  rhs=fcwel_sb[:, c * FH + F: (c + 1) * FH],
                                  start=(c == 0), stop=(c == 1))
                    pe.wait_ge(ohm[so], ohmark[n])
                    pe.wait_ge(DVW, w + 1)
                    # er_e = onehT.T @ erw_sb
                    pe.matmul(fc_ps[:, F + H: F + 2 * H],
                              lhsT=oh_sb[:, so * 256 + 128: (so + 1) * 256],
                              rhs=erw_sb[:, pw * H:(pw + 1) * H], start=True, stop=True).then_inc(PE2, 1)
                    pe.wait_ge(DVM, n + 1)
                    if t == 0 and w >= 1:
                        pe.wait_ge(DVR1, w)      # rst/den read by epilogue w-1
                    pe.matmul(rst_ps[:, :], lhsT=oh_sb[:, so * 256: so * 256 + 128],
                              rhs=msgs_sb[:, s2 * F:(s2 + 1) * F],
                              start=(t == 0), stop=(t == T - 1))
                    pe.matmul(den_ps[:, 0:H], lhsT=oh_sb[:, so * 256: so * 256 + 128],
                              rhs=expb_sb[:, s2 * H:(s2 + 1) * H],
                              start=(t == 0), stop=(t == T - 1)).then_inc(PED, 1)
                # erw for next window
                if w + 1 < W:
                    pe.wait_ge(DVW, w + 1)       # previous erw copied out
                    erw_mm(w + 1).then_inc(PEW, 1)
                else:
                    pe.sem_inc(PEW, 1)
                # res for this window (single resw bank; epilogue w-1 must have read it)
                pe.wait_ge(DVR2, w)
                for c in range(2):
                    pe.matmul(resw_ps[:, 0:F],
                              lhsT=ftd_sb[:, c * DPC + w * 128: c * DPC + (w + 1) * 128],
                              rhs=reswer_sb[:, c * FH: c * FH + F],
                              start=(c == 0), stop=(c == 1))
                # cf transposes (tpc slices in tp banks, parity = next tile's)
                pe.wait_ge(DVC1, w + 1)
                tb = tp[(w * T + T) % 2]
                pe.transpose(tb[:, 256:384], cft_sb[:, pw * IN: pw * IN + 128], ident_sb[:, :])
                pe.transpose(tb[:, 384:512], cft_sb[:, pw * IN + 128: (pw + 1) * IN], ident_sb[:, :]).then_inc(PETC, 1)
                pe.wait_ge(DVC2, w + 1)
                pe.wait_ge(DVR2, w)   # fd read of previous window's poid done
                # poid = cfT.T @ wwt -> fc_ps[:, 0:HID]
                for c in range(2):
                    mm = pe.matmul(fc_ps[:, 0:HID],
                              lhsT=cft_sb[:, pw * IN + c * 128: pw * IN + (c + 1) * 128],
                              rhs=wwt_sb[:, c * HID:(c + 1) * HID],
                              start=(c == 0), stop=(c == 1))
                mm.then_inc(PEP, 1)

        @block.vector
        def _(dv):
            A = mybir.AluOpType
            dv.wait_ge(ld, NPRE * 16)
            for w in range(W):
                pw = w % 2
                dv.wait_ge(PEW, w + 1)
                dv.tensor_copy(erw_sb[:, pw * H:(pw + 1) * H], resw_ps[:, F:FH]).then_inc(DVW, 1)
                for t in range(T):
                    n = w * T + t
                    s2 = n % 2
                    dv.wait_ge(PET, n + 1)
                    dv.tensor_copy(gt_sb[:, s2 * IN: s2 * IN + 128], tp[s2][:, 0:128])
                    dv.tensor_copy(gt_sb[:, s2 * IN + 128: (s2 + 1) * IN], tp[s2][:, 128:256]).then_inc(DVT, 1)
                    dv.wait_ge(AC1, n + 1)
                    dv.tensor_add(e_sb[:, s2 * H:(s2 + 1) * H], fc_ps[:, F:F + H],
                                  erf_sb[:, s2 * H:(s2 + 1) * H])
                    dv.drain()
                    dv.tensor_scalar_mul(lr_sb[:, s2 * H:(s2 + 1) * H],
                                         e_sb[:, s2 * H:(s2 + 1) * H], NEG)
                    dv.drain()
                    dv.tensor_tensor(out=lr_sb[:, s2 * H:(s2 + 1) * H],
                                     in0=lr_sb[:, s2 * H:(s2 + 1) * H],
                                     in1=e_sb[:, s2 * H:(s2 + 1) * H],
                                     op=A.max).then_inc(DVA, 1)
                    dv.wait_ge(AC2, n + 1)
                    dv.tensor_copy(expb_sb[:, s2 * H:(s2 + 1) * H], exp_sb[:, s2 * H:(s2 + 1) * H])
                    dv.drain()
                    dv.tensor_tensor(
                        out=msgs_sb[:, s2 * F:(s2 + 1) * F],
                        in0=fc_ps[:, 0:F],
                        in1=expb_sb[:, s2 * H:(s2 + 1) * H].rearrange("p (h o) -> p h o", o=1).to_broadcast([128, H, HID]),
                        op=A.mult).then_inc(DVM, 1)
                # poi cf cast -> bf16 (cft staging pre-transpose uses cft_sb as [d,k] bf16)
                dv.wait_ge(gpoi[w % 2], pmark[w])
                dv.tensor_copy(cft_sb[:, pw * IN:(pw + 1) * IN], poi_sb[:, pw * PW: pw * PW + IN]).then_inc(DVC1, 1)
                dv.wait_ge(PETC, w + 1)
                tb = tp[(w * T + T) % 2]
                dv.tensor_copy(cft_sb[:, pw * IN: pw * IN + 128], tb[:, 256:384])
                dv.tensor_copy(cft_sb[:, pw * IN + 128: (pw + 1) * IN], tb[:, 384:512]).then_inc(DVC2, 1)
                # ---- epilogue ----
                dv.wait_ge(PEP, w + 1)
                dv.tensor_scalar(out=dinv_sb[:, pw * H:(pw + 1) * H], in0=den_ps[:, 0:H],
                                 scalar1=1e-9, scalar2=None, op0=A.max)
                dv.drain()
                dv.reciprocal(dinv_sb[:, pw * H:(pw + 1) * H], dinv_sb[:, pw * H:(pw + 1) * H])
                dv.drain()
                dv.tensor_tensor(
                    out=z_sb[:, :], in0=rst_ps[:, :],
                    in1=dinv_sb[:, pw * H:(pw + 1) * H].rearrange("p (h o) -> p h o", o=1).to_broadcast([128, H, HID]),
                    op=A.mult).then_inc(DVR1, 1)
                dv.drain()
                dv.tensor_add(z_sb[:, :], z_sb[:, :], resw_ps[:, 0:F])
                dv.drain()
                dv.tensor_add(z_sb[:, :], z_sb[:, :], bias_sb[:, :])
                dv.drain()
                # s = sum(coeff * (iota < count))
                dv.tensor_scalar(out=mask_sb[:, :], in0=iota_sb[:, 0:64],
                                 scalar1=poi_sb[:, pw * PW + IN + 56: pw * PW + IN + 57],
                                 scalar2=None, op0=A.is_lt)
                dv.drain()
                dv.tensor_tensor(out=mask_sb[:, :], in0=mask_sb[:, :],
                                 in1=poi_sb[:, pw * PW + IN: pw * PW + IN + 64], op=A.mult)
                dv.drain()
                dv.tensor_reduce(out=s_sb[:, pw:pw + 1], in_=mask_sb[:, :],
                                 axis=mybir.AxisListType.X, op=A.add)
                dv.drain()
                dv.tensor_scalar(out=fd_sb[:, pw * HID:(pw + 1) * HID], in0=fc_ps[:, 0:HID],
                                 scalar1=s_sb[:, pw:pw + 1], scalar2=None, op0=A.mult).then_inc(DVR2, 1)
                dv.tensor_scalar(out=zn_sb[:, :], in0=z_sb[:, :],
                                 scalar1=0.0, scalar2=None, op0=A.min).then_inc(DVZ, 1)
                dv.wait_ge(AC3, w + 1)
                dv.tensor_add(q_sb[:, :], pos_sb[:, :], e1_sb[:, :])
                dv.drain()
                dv.tensor_tensor(
                    out=q_sb[:, :], in0=q_sb[:, :],
                    in1=fd_sb[:, pw * HID:(pw + 1) * HID].rearrange("p (o d) -> p o d", o=1).to_broadcast([128, H, HID]),
                    op=A.add).then_inc(DVQ, 1)

        @block.scalar
        def _(ac):
            AF = mybir.ActivationFunctionType
            ac.wait_ge(ld, NPRE * 16)
            for w in range(W):
                pw = w % 2
                for t in range(T):
                    n = w * T + t
                    s2 = n % 2
                    ac.wait_ge(PE2, n + 1)
                    ac.activation(erf_sb[:, s2 * H:(s2 + 1) * H], fc_ps[:, F + H:F + 2 * H], AF.Copy).then_inc(AC1, 1)
                    ac.wait_ge(DVA, n + 1)
                    ac.activation(exp_sb[:, s2 * H:(s2 + 1) * H], lr_sb[:, s2 * H:(s2 + 1) * H], AF.Exp).then_inc(AC2, 1)
                # epilogue
                ac.wait_ge(DVZ, w + 1)
                ac.activation(e1_sb[:, :], zn_sb[:, :], AF.Exp)
                ac.activation(pos_sb[:, :], z_sb[:, :], AF.Relu).then_inc(AC3, 1)
                ac.wait_ge(DVQ, w + 1)
                # y = relu(0.5*q - 0.5) ; (q = pos+exp(zn)+fd, elu+dist merged, -1 folded)
                ac.activation(outs_sb[:, pw * F:(pw + 1) * F], q_sb[:, :], AF.Relu,
                              bias=iota_sb[:, 64:65], scale=0.5).then_inc(AC4, 1)
                ac.wait_ge(AC4, w + 1)
                if w >= 2:
                    ac.wait_ge(osem[pw], 16 * (w // 2))
                ac.dma_start(out_d[w * 128:(w + 1) * 128, :], outs_sb[:, pw * F:(pw + 1) * F]).then_inc(osem[pw], 16)
            ac.wait_ge(osem[0], 16 * ((W + 1) // 2))
            ac.wait_ge(osem[1], 16 * (W // 2))

    return nc


def assemble(results, cfg: Cfg):
    parts = [np.asarray(r["out"]) for r in results]
    out = np.concatenate(parts, axis=0)              # [n_dst, F]
    return out.reshape(cfg.n_dst, cfg.heads, cfg.hid).astype(np.float32)


def numpy_emulate(inputs, cfg: Cfg):
    """Pure-numpy emulation of the kernel's math (fp32) for algorithm sanity."""
    feat = np.asarray(inputs["feat"], np.float32)
    fc_w = np.asarray(inputs["fc_w"], np.float32)
    attn_l = np.asarray(inputs["attn_l"], np.float32)
    attn_r = np.asarray(inputs["attn_r"], np.float32)
    bias_p = np.asarray(inputs["bias_p"], np.float32)
    res_w = np.asarray(inputs["res_w"], np.float32)
    w_w = np.asarray(inputs["w_w"], np.float32)
    src = np.asarray(inputs["src_idx"]).astype(np.int64)
    dst = np.asarray(inputs["dst_idx"]).astype(np.int64)
    onodes = np.asarray(inputs["output_nodes"]).astype(np.int64)
    ncounts = np.asarray(inputs["neighbor_counts"]).astype(np.int64)
    poi_cat = np.asarray(inputs["poi_cat_feat"], np.float32)
    poi_coe = np.asarray(inputs["poi_coeff"], np.float32)
    H, HID, IN = cfg.heads, cfg.hid, cfg.in_dim
    W_el = np.einsum("hfk,hf->kh", fc_w.reshape(H, HID, IN), attn_l)
    W_er = np.einsum("hfk,hf->kh", fc_w.reshape(H, HID, IN), attn_r)
    el = feat @ W_el
    er = feat[:cfg.n_dst] @ W_er
    fs = (feat @ fc_w.T).reshape(-1, H, HID)
    e = el[src] + er[dst]
    e = np.where(e > 0, e, cfg.neg_slope * e)
    ex = np.exp(e)
    den = np.zeros((cfg.n_dst, H), np.float32)
    np.add.at(den, dst, ex)
    rst = np.zeros((cfg.n_dst, H, HID), np.float32)
    np.add.at(rst, dst, ex[:, :, None] * fs[src])
    rst = rst / np.maximum(den, 1e-9)[:, :, None]
    res = (feat[:cfg.n_dst] @ res_w.T).reshape(-1, H, HID)
    z = rst + res + bias_p.reshape(1, H, HID)
    gat = np.where(z > 0, z, 0) + np.exp(np.minimum(z, 0)) - 1
    cf = poi_cat[onodes]
    valid = np.arange(cfg.k)[None, :] < ncounts[onodes][:, None]
    s = (poi_coe[onodes] * valid).sum(-1)
    fdist = (cf @ w_w.T) * s[:, None]
    return np.maximum((gat + fdist[:, None, :]) / 2, 0)


# ---------------- SPMD runner (jit-once, PJRT via axon) ----------------
import jax
from jax.sharding import Mesh, PartitionSpec
from jax.experimental.shard_map import shard_map
from concourse.bass2jax import _bass_exec_p, install_neuronx_cc_hook, partition_id_tensor


class SpmdRunner:
    def __init__(self, nc: bass.Bass, n_cores: int = 8):
        install_neuronx_cc_hook()
        self.nc = nc
        self.n_cores = n_cores
        pname0 = nc.partition_id_tensor.name if nc.partition_id_tensor else None
        in_names, out_names, out_avals = [], [], []
        for alloc in nc.m.functions[0].allocations:
            if not isinstance(alloc, mybir.MemoryLocationSet):
                continue
            name = alloc.memorylocations[0].name
            if alloc.kind == "ExternalInput":
                if name != pname0:
                    in_names.append(name)
            elif alloc.kind == "ExternalOutput":
                out_names.append(name)
                out_avals.append(jax.core.ShapedArray(tuple(alloc.tensor_shape), mybir.dt.np(alloc.dtype)))
        self.in_names, self.out_names, self.out_avals = in_names, out_names, out_avals
        n_params = len(in_names)
        n_outs = len(out_avals)
        pname = nc.partition_id_tensor.name if nc.partition_id_tensor else None
        all_names = in_names + out_names + ([pname] if pname else [])

        def _body(*args):
            operands = list(args)
            if pname is not None:
                operands.append(partition_id_tensor())
            outs = _bass_exec_p.bind(
                *operands,
                out_avals=tuple(out_avals),
                in_names=tuple(all_names),
                out_names=tuple(out_names),
                lowering_input_output_aliases=(),
                sim_require_finite=False,
                sim_require_nnan=False,
                nc=nc,
            )
            return tuple(outs)

        devices = jax.devices()[:n_cores]
        mesh = Mesh(np.asarray(devices), ("core",))
        in_specs = (PartitionSpec("core"),) * (n_params + n_outs)
        out_specs = (PartitionSpec("core"),) * n_outs
        self.donate = tuple(range(n_params, n_params + n_outs))
        self.fn = jax.jit(
            shard_map(_body, mesh=mesh, in_specs=in_specs, out_specs=out_specs, check_rep=False),
            donate_argnums=self.donate, keep_unused=True,
        )
        self.mesh = mesh

    def stage_inputs(self, in_maps):
        """Concat per-core inputs and device_put once. Returns list of device arrays."""
        per_core = [[np.asarray(m[name]) for name in self.in_names] for m in in_maps]
        concat_in = [np.concatenate([per_core[c][i] for c in range(self.n_cores)], axis=0)
                     for i in range(len(self.in_names))]
        self.staged = [jax.device_put(x) for x in concat_in]
        jax.block_until_ready(self.staged)
        return self.staged

    def _zeros(self):
        return [np.zeros((self.n_cores * a.shape[0], *a.shape[1:]), a.dtype) for a in self.out_avals]

    def run(self):
        outs = self.fn(*self.staged, *self._zeros())
        jax.block_until_ready(outs)
        return [
            {name: np.asarray(outs[i]).reshape(self.n_cores, *self.out_avals[i].shape)[c]
             for i, name in enumerate(self.out_names)}
            for c in range(self.n_cores)
        ]

    def time_runs(self, n=8):
        import time
        ts = []
        for _ in range(n):
            zeros = self._zeros()
            t0 = time.time()
            outs = self.fn(*self.staged, *zeros)
            jax.block_until_ready(outs)
            ts.append(time.time() - t0)
        return ts


_CACHE = {}


def kernel(**inputs) -> np.ndarray:
    cfg = Cfg()
    in_maps, T = host_prep(inputs, cfg)
    key = ("gat", T)
    if key not in _CACHE:
        nc = build_nc(cfg, T)
        _CACHE[key] = SpmdRunner(nc, cfg.cores)
    r = _CACHE[key]
    r.stage_inputs(in_maps)
    results = r.run()
    return assemble(results, cfg)



# revision 8
# speedup vs baseline: 235.5343x; 235.5343x over previous
# GAT message-passing kernel for 8 TRN2 NeuronCores (raw bass, SPMD).
#
# Strategy (dst-sharded, no collectives):
#  - Edges bucketed by dst into 8 cores x 16 windows (128 dsts each), padded
#    to T 128-edge tiles per window.
#  - Per tile: indirect-gather bf16 feat rows by src, PE-transpose, matmul
#    against [fc_w.T | W_el] (W_el = attn_l folded into fc_w so el comes from
#    the same matmul), edge coefficients exp(leaky(el+er)) via ACT, weighted
#    segment-sum via one-hot matmuls into PSUM per window.
#  - er/res per window from local feat.T columns; POI branch gathered from a
#    host-packed [cat|coeff|count] table; epilogue ELU + rescale on DVE/ACT.
import dataclasses
import numpy as np
import ml_dtypes

import concourse.bass as bass
from concourse import mybir

INC_G = 16   # sem inc per feat-row indirect gather (sim-calibrated)
INC_OH = 64  # sem inc per onehot-pair HWDGE load
INC_P = 16   # sem inc per poi indirect gather
BF16 = mybir.dt.bfloat16
F32 = mybir.dt.float32
I32 = mybir.dt.int32


@dataclasses.dataclass
class Cfg:
    n_src: int = 65536
    n_dst: int = 16384
    n_edges: int = 262144
    in_dim: int = 256          # must be 2*128
    hid: int = 128
    heads: int = 4
    n_poi: int = 100000
    k: int = 50
    neg_slope: float = 0.2
    cores: int = 8

    @property
    def f(self):  # total output features
        return self.heads * self.hid

    @property
    def dsts_per_core(self):
        return self.n_dst // self.cores

    @property
    def windows(self):
        return self.dsts_per_core // 128


def host_prep(inputs: dict, cfg: Cfg):
    """Build per-core input maps. Index prep / layout / dtype casts only."""
    bf = ml_dtypes.bfloat16
    feat = np.asarray(inputs["feat"], np.float32)
    poi_cat = np.asarray(inputs["poi_cat_feat"], np.float32)
    poi_coe = np.asarray(inputs["poi_coeff"], np.float32)
    fc_w = np.asarray(inputs["fc_w"], np.float32)
    attn_l = np.asarray(inputs["attn_l"], np.float32)
    attn_r = np.asarray(inputs["attn_r"], np.float32)
    bias_p = np.asarray(inputs["bias_p"], np.float32)
    res_w = np.asarray(inputs["res_w"], np.float32)
    w_w = np.asarray(inputs["w_w"], np.float32)
    src_idx = np.asarray(inputs["src_idx"]).astype(np.int64)
    dst_idx = np.asarray(inputs["dst_idx"]).astype(np.int64)
    out_nodes = np.asarray(inputs["output_nodes"]).astype(np.int64)
    ncounts = np.asarray(inputs["neighbor_counts"]).astype(np.int64)

    H, HID, IN, F, K = cfg.heads, cfg.hid, cfg.in_dim, cfg.f, cfg.k
    W, DPC = cfg.windows, cfg.dsts_per_core

    # --- weight folding (weights-only math) ---
    # el[s,h] = sum_k feat[s,k] * W_el[k,h];  W_el[k,h] = sum_f fc_w[h*HID+f,k]*attn_l[h,f]
    W_el = np.einsum("hfk,hf->kh", fc_w.reshape(H, HID, IN), attn_l)  # [IN, H]
    W_er = np.einsum("hfk,hf->kh", fc_w.reshape(H, HID, IN), attn_r)  # [IN, H]

    fcwel = np.concatenate([fc_w.T, W_el], axis=1)            # [IN, F+H]
    reswer = np.concatenate([res_w.T, W_er], axis=1)          # [IN, F+H]
    # pack [IN, X] as [128, 2*X]: chunk c -> cols [c*X:(c+1)*X]
    def pack_k(a):
        X = a.shape[1]
        return np.concatenate([a[0:128], a[128:256]], axis=1).astype(bf)

    fcwel_p = pack_k(fcwel)                                   # [128, 2*(F+H)]
    reswer_p = pack_k(reswer)                                 # [128, 2*(F+H)]
    wwT_p = pack_k(w_w.T)                                     # [128, 2*HID]

    featb = np.ascontiguousarray(feat).astype(bf)             # [n_src, IN]
    featT = feat.T.astype(bf)                                 # [IN, n_dst slice used]
    bias_b = np.tile(bias_p[None, :], (128, 1)).astype(np.float32)   # [128, F]
    iota = np.zeros((128, 66), np.float32)
    iota[:, :64] = np.arange(64, dtype=np.float32)[None, :]
    iota[:, 64] = -0.5
    ident = np.eye(128, dtype=bf)

    # POI combined table: [cat(IN) | coeff(K..pad64) | count | pad] -> 320 f32
    PW = IN + 64
    poitab = np.zeros((cfg.n_poi, PW), np.float32)
    poitab[:, :IN] = poi_cat
    poitab[:, IN:IN + K] = poi_coe
    poitab[:, IN + 56] = ncounts.astype(np.float32)           # count at col IN+56
    assert K <= 56

    # --- edge bucketing ---
    order = np.argsort(dst_idx, kind="stable")
    ds = dst_idx[order]
    # counts per (core, window)
    win_of = ds // 128                                        # global window id
    n_win_glob = cfg.n_dst // 128
    cnt = np.bincount(win_of, minlength=n_win_glob)
    T = int(np.ceil(cnt.max() / 128))
    starts = np.zeros(n_win_glob + 1, np.int64)
    np.cumsum(cnt, out=starts[1:])

    in_maps = []
    for c in range(cfg.cores):
        gidx = np.zeros((128, W * T), np.int32)
        dloc = np.full((W * T, 128), 255, np.int32)           # 255 = pad slot
        for w in range(W):
            gw = c * W + w
            e0, e1 = starts[gw], starts[gw + 1]
            sel = order[e0:e1]
            nme = e1 - e0
            s_pad = np.zeros(T * 128, np.int64)
            d_pad = np.full(T * 128, 255, np.int64)
            s_pad[:nme] = src_idx[sel]
            d_pad[:nme] = dst_idx[sel] % 128
            for t in range(T):
                n = w * T + t
                gidx[:, n] = s_pad[t * 128:(t + 1) * 128]
                dloc[n] = d_pad[t * 128:(t + 1) * 128]
        # one-hot [NT,128,128] + transpose, packed as [NT,128,256]
        oh = (dloc[:, :, None] == np.arange(128)[None, None, :])
        ohcomb = np.zeros((W * T, 128, 256), bf)
        ohcomb[:, :, :128] = oh.astype(bf)
        ohcomb[:, :, 128:] = np.transpose(oh, (0, 2, 1)).astype(bf)

        poix = np.zeros((128, 16), np.int32)
        poix[:, :W] = out_nodes[c * DPC:(c + 1) * DPC].reshape(W, 128).T

        fTd = featT[:, c * DPC:(c + 1) * DPC]                 # [IN, DPC]
        fTd_p = np.concatenate([fTd[0:128], fTd[128:256]], axis=1).astype(bf)  # [128, 2*DPC]

        in_maps.append(dict(
            featb=featb, poitab=poitab,
            gidx=gidx, ohcomb=np.ascontiguousarray(ohcomb), poix=poix,
            fcwel=fcwel_p, reswer=reswer_p, wwt=wwT_p, ftd=fTd_p,
            biasb=bias_b, iota=iota, ident=ident,
        ))
    return in_maps, T


def build_nc(cfg: Cfg, T: int):
    H, HID, IN, F, K = cfg.heads, cfg.hid, cfg.in_dim, cfg.f, cfg.k
    NEG = cfg.neg_slope
    W, DPC = cfg.windows, cfg.dsts_per_core
    NT = W * T
    FH = F + H                     # 516
    GB, OH = 4, 4                  # gather / onehot slot counts
    IG = INC_G
    IP = INC_P
    IO = INC_OH
    PW = IN + 64                   # poi row width (f32)

    nc = bass.Bass(num_devices=cfg.cores)
    # DRAM inputs
    featb = nc.dram_tensor("featb", [cfg.n_src, IN], BF16, kind="ExternalInput")
    poitab = nc.dram_tensor("poitab", [cfg.n_poi, PW], F32, kind="ExternalInput")
    gidx = nc.dram_tensor("gidx", [128, NT], I32, kind="ExternalInput")
    ohcomb = nc.dram_tensor("ohcomb", [NT, 128, 256], BF16, kind="ExternalInput")
    poix = nc.dram_tensor("poix", [128, 16], I32, kind="ExternalInput")
    fcwel = nc.dram_tensor("fcwel", [128, 2 * FH], BF16, kind="ExternalInput")
    reswer = nc.dram_tensor("reswer", [128, 2 * FH], BF16, kind="ExternalInput")
    wwt = nc.dram_tensor("wwt", [128, 2 * HID], BF16, kind="ExternalInput")
    ftd = nc.dram_tensor("ftd", [128, 2 * DPC], BF16, kind="ExternalInput")
    biasb = nc.dram_tensor("biasb", [128, F], F32, kind="ExternalInput")
    iota = nc.dram_tensor("iota", [128, 66], F32, kind="ExternalInput")
    ident = nc.dram_tensor("ident", [128, 128], BF16, kind="ExternalInput")
    out_d = nc.dram_tensor("out", [DPC, F], F32, kind="ExternalOutput")

    NPRE = 9   # preload DMA count (all but featb/poitab/ohcomb which stream)

    from contextlib import ExitStack
    ctx = ExitStack()
    idx_sb = ctx.enter_context(nc.sbuf_tensor([128, NT], I32))
    poix_sb = ctx.enter_context(nc.sbuf_tensor([128, 16], I32))
    fcwel_sb = ctx.enter_context(nc.sbuf_tensor([128, 2 * FH], BF16))
    reswer_sb = ctx.enter_context(nc.sbuf_tensor([128, 2 * FH], BF16))
    wwt_sb = ctx.enter_context(nc.sbuf_tensor([128, 2 * HID], BF16))
    ftd_sb = ctx.enter_context(nc.sbuf_tensor([128, 2 * DPC], BF16))
    bias_sb = ctx.enter_context(nc.sbuf_tensor([128, F], F32))
    iota_sb = ctx.enter_context(nc.sbuf_tensor([128, 66], F32))
    ident_sb = ctx.enter_context(nc.sbuf_tensor([128, 128], BF16))
    g_sb = ctx.enter_context(nc.sbuf_tensor([128, GB * IN], BF16))
    oh_sb = ctx.enter_context(nc.sbuf_tensor([128, OH * 256], BF16))
    gt_sb = ctx.enter_context(nc.sbuf_tensor([128, 2 * IN], BF16))
    erf_sb = ctx.enter_context(nc.sbuf_tensor([128, 2 * H], F32))
    e_sb = ctx.enter_context(nc.sbuf_tensor([128, 2 * H], F32))
    lr_sb = ctx.enter_context(nc.sbuf_tensor([128, 2 * H], F32))
    exp_sb = ctx.enter_context(nc.sbuf_tensor([128, 2 * H], F32))
    expb_sb = ctx.enter_context(nc.sbuf_tensor([128, 2 * H], BF16))
    erw_sb = ctx.enter_context(nc.sbuf_tensor([128, 2 * H], BF16))
    msgs_sb = ctx.enter_context(nc.sbuf_tensor([128, 2 * F], BF16))
    poi_sb = ctx.enter_context(nc.sbuf_tensor([128, 2 * PW], F32))
    cft_sb = ctx.enter_context(nc.sbuf_tensor([128, 2 * IN], BF16))
    s_sb = ctx.enter_context(nc.sbuf_tensor([128, 2], F32))
    mask_sb = ctx.enter_context(nc.sbuf_tensor([128, 64], F32))
    dinv_sb = ctx.enter_context(nc.sbuf_tensor([128, 2 * H], F32))
    z_sb = ctx.enter_context(nc.sbuf_tensor([128, F], F32))
    zn_sb = ctx.enter_context(nc.sbuf_tensor([128, F], F32))
    e1_sb = ctx.enter_context(nc.sbuf_tensor([128, F], F32))
    pos_sb = ctx.enter_context(nc.sbuf_tensor([128, F], F32))
    q_sb = ctx.enter_context(nc.sbuf_tensor([128, F], F32))
    fd_sb = ctx.enter_context(nc.sbuf_tensor([128, 2 * HID], F32))
    outs_sb = ctx.enter_context(nc.sbuf_tensor([128, 2 * F], F32))
    rst_ps = ctx.enter_context(nc.psum_tensor([128, F], F32))
    den_ps = ctx.enter_context(nc.psum_tensor([128, F], F32))
    resw_ps = ctx.enter_context(nc.psum_tensor([128, FH], F32))
    fc_ps = ctx.enter_context(nc.psum_tensor([128, F + 2 * H], F32))
    tpA_ps = ctx.enter_context(nc.psum_tensor([128, 1024], BF16))
    tpB_ps = ctx.enter_context(nc.psum_tensor([128, 1024], BF16))
    ld = ctx.enter_context(nc.semaphore())
    ohm = [ctx.enter_context(nc.semaphore(name=f"ohm{i}")) for i in range(4)]
    gsem = [ctx.enter_context(nc.semaphore(name=f"gs{i}")) for i in range(4)]
    gpoi = [ctx.enter_context(nc.semaphore(name=f"gp{i}")) for i in range(2)]
    PEW = ctx.enter_context(nc.semaphore())
    PET = ctx.enter_context(nc.semaphore())
    PE2 = ctx.enter_context(nc.semaphore())
    PED = ctx.enter_context(nc.semaphore())
    PETC = ctx.enter_context(nc.semaphore())
    PEP = ctx.enter_context(nc.semaphore())
    DVW = ctx.enter_context(nc.semaphore())
    DVT = ctx.enter_context(nc.semaphore())
    DVA = ctx.enter_context(nc.semaphore())
    DVM = ctx.enter_context(nc.semaphore())
    DVC1 = ctx.enter_context(nc.semaphore())
    DVC2 = ctx.enter_context(nc.semaphore())
    DVR1 = ctx.enter_context(nc.semaphore())
    DVR2 = ctx.enter_context(nc.semaphore())
    DVZ = ctx.enter_context(nc.semaphore())
    DVQ = ctx.enter_context(nc.semaphore())
    AC1 = ctx.enter_context(nc.semaphore())
    AC2 = ctx.enter_context(nc.semaphore())
    AC3 = ctx.enter_context(nc.semaphore())
    AC4 = ctx.enter_context(nc.semaphore())
    ACP = ctx.enter_context(nc.semaphore())
    osem = [ctx.enter_context(nc.semaphore(name=f"os{i}")) for i in range(2)]
    block = ctx.enter_context(nc.Block())
    with ctx:
        tp = [tpA_ps, tpB_ps]

        def gpos(w, t):  # gather sequence position of tile (w,t); poi w at w*(T+1)
            return w * (T + 1) + 1 + t

        ohmark = {}

        @block.sync
        def _(sy):
            sy.dma_start(idx_sb[:, :], gidx[:, :]).then_inc(ld, 16)
            sy.dma_start(poix_sb[:, :], poix[:, :]).then_inc(ld, 16)
            sy.dma_start(fcwel_sb[:, :], fcwel[:, :]).then_inc(ld, 16)
            sy.dma_start(reswer_sb[:, :], reswer[:, :]).then_inc(ld, 16)
            sy.dma_start(wwt_sb[:, :], wwt[:, :]).then_inc(ld, 16)
            sy.dma_start(ftd_sb[:, :], ftd[:, :]).then_inc(ld, 16)
            sy.dma_start(bias_sb[:, :], biasb[:, :]).then_inc(ld, 16)
            sy.dma_start(iota_sb[:, :], iota[:, :]).then_inc(ld, 16)
            sy.dma_start(ident_sb[:, :], ident[:, :]).then_inc(ld, 16)
            # one-hot loads batched in pairs (one DMA covers tiles 2m, 2m+1)
            for m in range((NT + 1) // 2):
                n0 = 2 * m
                npair = min(2, NT - n0)
                if n0 + npair - 1 >= OH:
                    sy.wait_ge(PED, n0 + npair - OH)
                s = n0 % OH
                sy.dma_start(
                    oh_sb[:, s * 256:(s + npair) * 256].rearrange("p (g c) -> p g c", g=npair),
                    ohcomb[n0:n0 + npair, :, :].rearrange("g p c -> p g c"),
                ).then_inc(ohm[s], 16)
                for j in range(npair):
                    ohmark[n0 + j] = (s, (m // 2 + 1) * 16)

        gsem_val = [0]
        gmark = {}
        pmark = {}

        @block.gpsimd
        def _(gp):
            gp.wait_ge(ld, NPRE * 16)
            for w in range(W):
                if w >= 2:
                    gp.wait_ge(DVR2, w - 1)  # poi slot free
                pw = w % 2
                gp.indirect_dma_start(
                    out=poi_sb[:, pw * PW:(pw + 1) * PW], out_offset=None,
                    in_=poitab[:, :],
                    in_offset=bass.IndirectOffsetOnAxis(ap=poix_sb[:, w:w + 1], axis=0),
                ).then_inc(gpoi[w % 2], 16)
                pmark[w] = (w // 2 + 1) * 16
                for t in range(T):
                    n = w * T + t
                    if n >= GB:
                        gp.wait_ge(PET, n - GB + 1)  # gather slot free after transposes
                    s = n % GB
                    gp.indirect_dma_start(
                        out=g_sb[:, s * IN:(s + 1) * IN], out_offset=None,
                        in_=featb[:, :],
                        in_offset=bass.IndirectOffsetOnAxis(ap=idx_sb[:, n:n + 1], axis=0),
                    ).then_inc(gsem[s], 16)
                    gmark[(w, t)] = (n // GB + 1) * IG

        @block.tensor
        def _(pe):
            pe.wait_ge(ld, NPRE * 16)

            def erw_mm(w):
                # erw(w) into resw_ps[:, F:FH]; returns last matmul
                last = None
                for c in range(2):
                    last = pe.matmul(resw_ps[:, F:FH],
                              lhsT=ftd_sb[:, c * DPC + w * 128: c * DPC + (w + 1) * 128],
                              rhs=reswer_sb[:, c * FH + F: c * FH + FH],
                              start=(c == 0), stop=(c == 1))
                return last

            erw_mm(0).then_inc(PEW, 1)
            for w in range(W):
                pw = w % 2
                for t in range(T):
                    n = w * T + t
                    s2 = n % 2
                    sg = n % GB
                    so = n % OH
                    pe.wait_ge(gsem[n % GB], gmark[(w, t)])
                    if t == 0 and w > 0:
                        pe.wait_ge(DVC2, w)      # tp bank shared with tpc of prev window
                    # transposes of gathered tile (bf16 -> psum bf16)
                    pe.transpose(tp[s2][:, 0:128], g_sb[:, sg * IN: sg * IN + 128], ident_sb[:, :])
                    pe.transpose(tp[s2][:, 128:256], g_sb[:, sg * IN + 128: (sg + 1) * IN], ident_sb[:, :]).then_inc(PET, 1)
                    pe.wait_ge(DVT, n + 1)
                    # fc + el
                    for c in range(2):
                        lhs = gt_sb[:, s2 * IN + c * 128: s2 * IN + (c + 1) * 128]
                        pe.matmul(fc_ps[:, 0:F], lhsT=lhs,
                                  rhs=fcwel_sb[:, c * FH: c * FH + F],
                                  start=(c == 0), stop=(c == 1))
                        pe.matmul(fc_ps[:, F:F + H], lhsT=lhs,
                                  rhs=fcwel_sb[:, c * FH + F: (c + 1) * FH],
                                  start=(c == 0), stop=(c == 1))
                    pe.wait_ge(ohm[ohmark[n][0]], ohmark[n][1])
                    pe.wait_ge(DVW, w + 1)
                    # er_e = onehT.T @ erw_sb
                    pe.matmul(fc_ps[:, F + H: F + 2 * H],
                              lhsT=oh_sb[:, so * 256 + 128: (so + 1) * 256],
                              rhs=erw_sb[:, pw * H:(pw + 1) * H], start=True, stop=True).then_inc(PE2, 1)
                    pe.wait_ge(DVM, n + 1)
                    if t == 0 and w >= 1:
                        pe.wait_ge(DVR1, w)      # rst/den read by epilogue w-1
                    pe.matmul(rst_ps[:, :], lhsT=oh_sb[:, so * 256: so * 256 + 128],
                              rhs=msgs_sb[:, s2 * F:(s2 + 1) * F],
                              start=(t == 0), stop=(t == T - 1))
                    pe.matmul(den_ps[:, 0:H], lhsT=oh_sb[:, so * 256: so * 256 + 128],
                              rhs=expb_sb[:, s2 * H:(s2 + 1) * H],
                              start=(t == 0), stop=(t == T - 1)).then_inc(PED, 1)
                # erw for next window
                if w + 1 < W:
                    pe.wait_ge(DVW, w + 1)       # previous erw copied out
                    erw_mm(w + 1).then_inc(PEW, 1)
                else:
                    pe.sem_inc(PEW, 1)
                # res for this window (single resw bank; epilogue w-1 must have read it)
                pe.wait_ge(DVR2, w)
                for c in range(2):
                    pe.matmul(resw_ps[:, 0:F],
                              lhsT=ftd_sb[:, c * DPC + w * 128: c * DPC + (w + 1) * 128],
                              rhs=reswer_sb[:, c * FH: c * FH + F],
                              start=(c == 0), stop=(c == 1))
                # cf transposes (tpc slices in tp banks, parity = next tile's)
                pe.wait_ge(DVC1, w + 1)
                tb = tp[(w * T + T) % 2]
                pe.transpose(tb[:, 256:384], cft_sb[:, pw * IN: pw * IN + 128], ident_sb[:, :])
                pe.transpose(tb[:, 384:512], cft_sb[:, pw * IN + 128: (pw + 1) * IN], ident_sb[:, :]).then_inc(PETC, 1)
                pe.wait_ge(DVC2, w + 1)
                pe.wait_ge(DVR2, w)   # fd read of previous window's poid done
                # poid = cfT.T @ wwt -> fc_ps[:, 0:HID]
                for c in range(2):
                    mm = pe.matmul(fc_ps[:, 0:HID],
                              lhsT=cft_sb[:, pw * IN + c * 128: pw * IN + (c + 1) * 128],
                              rhs=wwt_sb[:, c * HID:(c + 1) * HID],
                              start=(c == 0), stop=(c == 1))
                mm.then_inc(PEP, 1)

        @block.vector
        def _(dv):
            A = mybir.AluOpType
            dv.wait_ge(ld, NPRE * 16)
            for w in range(W):
                pw = w % 2
                dv.wait_ge(PEW, w + 1)
                dv.tensor_copy(erw_sb[:, pw * H:(pw + 1) * H], resw_ps[:, F:FH]).then_inc(DVW, 1)
                for t in range(T):
                    n = w * T + t
                    s2 = n % 2
                    dv.wait_ge(PET, n + 1)
                    dv.tensor_copy(gt_sb[:, s2 * IN: s2 * IN + 128], tp[s2][:, 0:128])
                    dv.tensor_copy(gt_sb[:, s2 * IN + 128: (s2 + 1) * IN], tp[s2][:, 128:256]).then_inc(DVT, 1)
                    dv.wait_ge(AC1, n + 1)
                    dv.tensor_add(e_sb[:, s2 * H:(s2 + 1) * H], fc_ps[:, F:F + H],
                                  erf_sb[:, s2 * H:(s2 + 1) * H]).then_inc(DVA, 1)
                    dv.wait_ge(AC2, n + 1)
                    dv.tensor_tensor(
                        out=msgs_sb[:, s2 * F:(s2 + 1) * F],
                        in0=fc_ps[:, 0:F],
                        in1=expb_sb[:, s2 * H:(s2 + 1) * H].rearrange("p (h o) -> p h o", o=1).to_broadcast([128, H, HID]),
                        op=A.mult).then_inc(DVM, 1)
                # poi cf cast -> bf16 (cft staging pre-transpose uses cft_sb as [d,k] bf16)
                dv.wait_ge(gpoi[w % 2], pmark[w])
                dv.tensor_copy(cft_sb[:, pw * IN:(pw + 1) * IN], poi_sb[:, pw * PW: pw * PW + IN]).then_inc(DVC1, 1)
                dv.wait_ge(PETC, w + 1)
                tb = tp[(w * T + T) % 2]
                dv.tensor_copy(cft_sb[:, pw * IN: pw * IN + 128], tb[:, 256:384])
                dv.tensor_copy(cft_sb[:, pw * IN + 128: (pw + 1) * IN], tb[:, 384:512]).then_inc(DVC2, 1)
                # ---- epilogue ----
                dv.wait_ge(PEP, w + 1)
                dv.tensor_scalar(out=dinv_sb[:, pw * H:(pw + 1) * H], in0=den_ps[:, 0:H],
                                 scalar1=1e-9, scalar2=None, op0=A.max)
                dv.drain()
                dv.reciprocal(dinv_sb[:, pw * H:(pw + 1) * H], dinv_sb[:, pw * H:(pw + 1) * H])
                dv.drain()
                dv.tensor_tensor(
                    out=z_sb[:, :], in0=rst_ps[:, :],
                    in1=dinv_sb[:, pw * H:(pw + 1) * H].rearrange("p (h o) -> p h o", o=1).to_broadcast([128, H, HID]),
                    op=A.mult).then_inc(DVR1, 1)
                dv.drain()
                dv.tensor_add(z_sb[:, :], z_sb[:, :], resw_ps[:, 0:F])
                dv.drain()
                dv.tensor_add(z_sb[:, :], z_sb[:, :], bias_sb[:, :])
                dv.drain()
                # s = sum(coeff * (iota < count))
                dv.tensor_scalar(out=mask_sb[:, :], in0=iota_sb[:, 0:64],
                                 scalar1=poi_sb[:, pw * PW + IN + 56: pw * PW + IN + 57],
                                 scalar2=None, op0=A.is_lt)
                dv.drain()
                dv.tensor_tensor(out=mask_sb[:, :], in0=mask_sb[:, :],
                                 in1=poi_sb[:, pw * PW + IN: pw * PW + IN + 64], op=A.mult)
                dv.drain()
                dv.tensor_reduce(out=s_sb[:, pw:pw + 1], in_=mask_sb[:, :],
                                 axis=mybir.AxisListType.X, op=A.add)
                dv.drain()
                dv.tensor_scalar(out=fd_sb[:, pw * HID:(pw + 1) * HID], in0=fc_ps[:, 0:HID],
                                 scalar1=s_sb[:, pw:pw + 1], scalar2=None, op0=A.mult).then_inc(DVR2, 1)
                dv.tensor_scalar(out=zn_sb[:, :], in0=z_sb[:, :],
                                 scalar1=0.0, scalar2=None, op0=A.min).then_inc(DVZ, 1)
                dv.wait_ge(AC3, w + 1)
                dv.tensor_add(q_sb[:, :], pos_sb[:, :], e1_sb[:, :])
                dv.drain()
                dv.tensor_tensor(
                    out=q_sb[:, :], in0=q_sb[:, :],
                    in1=fd_sb[:, pw * HID:(pw + 1) * HID].rearrange("p (o d) -> p o d", o=1).to_broadcast([128, H, HID]),
                    op=A.add).then_inc(DVQ, 1)

        @block.scalar
        def _(ac):
            AF = mybir.ActivationFunctionType
            ac.wait_ge(ld, NPRE * 16)
            for w in range(W):
                pw = w % 2
                for t in range(T):
                    n = w * T + t
                    s2 = n % 2
                    ac.wait_ge(PE2, n + 1)
                    ac.activation(erf_sb[:, s2 * H:(s2 + 1) * H], fc_ps[:, F + H:F + 2 * H], AF.Copy).then_inc(AC1, 1)
                    ac.wait_ge(DVA, n + 1)
                    # leaky relu on ACT (Prelu alpha), then exp -> bf16 directly
                    ac.activation(lr_sb[:, s2 * H:(s2 + 1) * H], e_sb[:, s2 * H:(s2 + 1) * H],
                                  AF.Prelu, alpha=NEG).then_inc(ACP, 1)
                    ac.wait_ge(ACP, n + 1)
                    ac.activation(expb_sb[:, s2 * H:(s2 + 1) * H], lr_sb[:, s2 * H:(s2 + 1) * H],
                                  AF.Exp).then_inc(AC2, 1)
                # epilogue
                ac.wait_ge(DVZ, w + 1)
                ac.activation(e1_sb[:, :], zn_sb[:, :], AF.Exp)
                ac.activation(pos_sb[:, :], z_sb[:, :], AF.Relu).then_inc(AC3, 1)
                ac.wait_ge(DVQ, w + 1)
                # y = relu(0.5*q - 0.5) ; (q = pos+exp(zn)+fd, elu+dist merged, -1 folded)
                ac.activation(outs_sb[:, pw * F:(pw + 1) * F], q_sb[:, :], AF.Relu,
                              bias=iota_sb[:, 64:65], scale=0.5).then_inc(AC4, 1)
                ac.wait_ge(AC4, w + 1)
                if w >= 2:
                    ac.wait_ge(osem[pw], 16 * (w // 2))
                ac.dma_start(out_d[w * 128:(w + 1) * 128, :], outs_sb[:, pw * F:(pw + 1) * F]).then_inc(osem[pw], 16)
            ac.wait_ge(osem[0], 16 * ((W + 1) // 2))
            ac.wait_ge(osem[1], 16 * (W // 2))

    return nc


def assemble(results, cfg: Cfg):
    parts = [np.asarray(r["out"]) for r in results]
    out = np.concatenate(parts, axis=0)              # [n_dst, F]
    return out.reshape(cfg.n_dst, cfg.heads, cfg.hid).astype(np.float32)


def numpy_emulate(inputs, cfg: Cfg):
    """Pure-numpy emulation of the kernel's math (fp32) for algorithm sanity."""
    feat = np.asarray(inputs["feat"], np.float32)
    fc_w = np.asarray(inputs["fc_w"], np.float32)
    attn_l = np.asarray(inputs["attn_l"], np.float32)
    attn_r = np.asarray(inputs["attn_r"], np.float32)
    bias_p = np.asarray(inputs["bias_p"], np.float32)
    res_w = np.asarray(inputs["res_w"], np.float32)
    w_w = np.asarray(inputs["w_w"], np.float32)
    src = np.asarray(inputs["src_idx"]).astype(np.int64)
    dst = np.asarray(inputs["dst_idx"]).astype(np.int64)
    onodes = np.asarray(inputs["output_nodes"]).astype(np.int64)
    ncounts = np.asarray(inputs["neighbor_counts"]).astype(np.int64)
    poi_cat = np.asarray(inputs["poi_cat_feat"], np.float32)
    poi_coe = np.asarray(inputs["poi_coeff"], np.float32)
    H, HID, IN = cfg.heads, cfg.hid, cfg.in_dim
    W_el = np.einsum("hfk,hf->kh", fc_w.reshape(H, HID, IN), attn_l)
    W_er = np.einsum("hfk,hf->kh", fc_w.reshape(H, HID, IN), attn_r)
    el = feat @ W_el
    er = feat[:cfg.n_dst] @ W_er
    fs = (feat @ fc_w.T).reshape(-1, H, HID)
    e = el[src] + er[dst]
    e = np.where(e > 0, e, cfg.neg_slope * e)
    ex = np.exp(e)
    den = np.zeros((cfg.n_dst, H), np.float32)
    np.add.at(den, dst, ex)
    rst = np.zeros((cfg.n_dst, H, HID), np.float32)
    np.add.at(rst, dst, ex[:, :, None] * fs[src])
    rst = rst / np.maximum(den, 1e-9)[:, :, None]
    res = (feat[:cfg.n_dst] @ res_w.T).reshape(-1, H, HID)
    z = rst + res + bias_p.reshape(1, H, HID)
    gat = np.where(z > 0, z, 0) + np.exp(np.minimum(z, 0)) - 1
    cf = poi_cat[onodes]
    valid = np.arange(cfg.k)[None, :] < ncounts[onodes][:, None]
    s = (poi_coe[onodes] * valid).sum(-1)
    fdist = (cf @ w_w.T) * s[:, None]
    return np.maximum((gat + fdist[:, None, :]) / 2, 0)


# ---------------- SPMD runner (jit-once, PJRT via axon) ----------------
import jax
from jax.sharding import Mesh, PartitionSpec
from jax.experimental.shard_map import shard_map
from concourse.bass2jax import (
    _bass_exec_p, install_neuronx_cc_hook, partition_id_tensor,
    fast_dispatch_compile,
)


class SpmdRunner:
    """Outputs are device-allocated by the NEFF (no zero-buffer operands);
    compiled with bass_effect suppressed for C++ fast-path dispatch."""

    def __init__(self, nc: bass.Bass, n_cores: int = 8):
        install_neuronx_cc_hook()
        self.nc = nc
        self.n_cores = n_cores
        pname = nc.partition_id_tensor.name if nc.partition_id_tensor else None
        in_names, out_names, out_avals = [], [], []
        for alloc in nc.m.functions[0].allocations:
            if not isinstance(alloc, mybir.MemoryLocationSet):
                continue
            name = alloc.memorylocations[0].name
            if alloc.kind == "ExternalInput":
                if name != pname:
                    in_names.append(name)
            elif alloc.kind == "ExternalOutput":
                out_names.append(name)
                out_avals.append(jax.core.ShapedArray(tuple(alloc.tensor_shape), mybir.dt.np(alloc.dtype)))
        self.in_names, self.out_names, self.out_avals = in_names, out_names, out_avals
        all_names = in_names + ([pname] if pname else [])

        def _body(*args):
            operands = list(args)
            if pname is not None:
                operands.append(partition_id_tensor())
            outs = _bass_exec_p.bind(
                *operands,
                out_avals=tuple(out_avals),
                in_names=tuple(all_names),
                out_names=tuple(out_names),
                lowering_input_output_aliases=(),
                sim_require_finite=False,
                sim_require_nnan=False,
                nc=nc,
            )
            return tuple(outs)

        devices = jax.devices()[:n_cores]
        mesh = Mesh(np.asarray(devices), ("core",))
        in_specs = (PartitionSpec("core"),) * len(in_names)
        out_specs = (PartitionSpec("core"),) * len(out_avals)
        self.fn = jax.jit(
            shard_map(_body, mesh=mesh, in_specs=in_specs, out_specs=out_specs, check_rep=False),
            keep_unused=True,
        )
        self.mesh = mesh
        self.compiled = None

    def stage_inputs(self, in_maps):
        """Concat per-core inputs and device_put once. Returns list of device arrays."""
        per_core = [[np.asarray(m[name]) for name in self.in_names] for m in in_maps]
        concat_in = [np.concatenate([per_core[c][i] for c in range(self.n_cores)], axis=0)
                     for i in range(len(self.in_names))]
        self.staged = [jax.device_put(x) for x in concat_in]
        jax.block_until_ready(self.staged)
        if self.compiled is None:
            self.compiled = fast_dispatch_compile(
                lambda: self.fn.lower(*self.staged).compile())
        return self.staged

    def call(self):
        return self.compiled(*self.staged)

    def run(self):
        outs = self.call()
        jax.block_until_ready(outs)
        return [
            {name: np.asarray(outs[i]).reshape(self.n_cores, *self.out_avals[i].shape)[c]
             for i, name in enumerate(self.out_names)}
            for c in range(self.n_cores)
        ]

    def time_runs(self, n=8):
        import time
        ts = []
        for _ in range(n):
            t0 = time.time()
            outs = self.call()
            jax.block_until_ready(outs)
            ts.append(time.time() - t0)
        return ts


_CACHE = {}


def kernel(**inputs) -> np.ndarray:
    cfg = Cfg()
    in_maps, T = host_prep(inputs, cfg)
    key = ("gat", T)
    if key not in _CACHE:
        nc = build_nc(cfg, T)
        _CACHE[key] = SpmdRunner(nc, cfg.cores)
    r = _CACHE[key]
    r.stage_inputs(in_maps)
    results = r.run()
    return assemble(results, cfg)

